# revision 46
# baseline (speedup 1.0000x reference)
"""CGConv (gnn_message_passing) Trainium2 kernel — 8-core SPMD, v3.

Strategy vs v2:
- x[dst] is gathered ON HOST into the sorted edge layout and streamed as a
  dense bf16 [128, e_pad] input (sequential DMA) — removes the SWDGE
  dma_gather whose GpSimd descriptor generation was 100%-busy for ~1ms.
- dst-half split dropped (only existed for int16 gather indices).
- slb row-replicate [128, e_pad] fp16 is built on host too: one-hot
  transposes come from a DVE is_equal against a streamed SBUF tile, killing
  all PE broadcast-transposes and the psum ping-pong they caused.
- Phase-1 batches of 1024 edges with double-buffered PSUM (2 halves x 2
  bufs x 2 banks = 8 banks) so psum drains overlap the next batch matmuls.
- BN1 sums come free: Act Copy accum_out (h0) + Pool scalar_tensor_tensor
  copy (h1); sum-of-squares via DVE scalar_tensor_tensor on bf16 stages.
- Phase 2 uses the native Softplus activation (no Exp+Ln) and groups
  activations per table set to minimize ACT_TABLE_LOAD switches.
- BN2 stats moved out of the scatter into a tiny end pass over `summed`.
"""

import sys

sys.path.insert(0, "/opt/trn_rl_repo")

import numpy as np
import ml_dtypes

from concourse import bass, bacc, tile, mybir
from concourse import bass_utils

BF16 = ml_dtypes.bfloat16
FP16 = np.float16
FP8 = ml_dtypes.float8_e4m3

# Problem constants (hardcoded per harness contract)
N, E, ATOM, NBR = 50000, 800000, 128, 64
DOUT = 2 * ATOM  # 256
BN_EPS = 1e-5

NCORES = 8
GPC = 49                      # groups of 128 nodes per core
NPC = GPC * 128               # 6272 nodes per core
NPAD = NCORES * NPC           # 50176
GB = 1024                     # phase-1 batch (edges)
MB = 4096                     # phase-2 mega-batch (edges)

_DT = mybir.dt


def _ceil(a, b):
    return -(-a // b)


def _prep(x, edge_index, edge_attr):
    """Host-side sharding: sort edges by (src core, src group), gather x[dst]
    into that layout, pad slots to uniform-across-cores sizes."""
    src = np.asarray(edge_index[0], dtype=np.int64)
    dst = np.asarray(edge_index[1], dtype=np.int64)
    ea = np.asarray(edge_attr, dtype=np.float32)

    g = src >> 7                      # node group 0..391
    core = g // GPC
    gi = g % GPC
    key = core * GPC + gi
    order = np.argsort(key, kind="stable")
    src_s, dst_s = src[order], dst[order]
    ea_s = ea[order].astype(BF16)

    counts = np.bincount(key, minlength=NCORES * GPC).reshape(NCORES, GPC)
    S = counts.max(axis=0)                      # [GPC] uniform slot sizes
    e_pad = _ceil(max(int(S.sum()), 1), GB) * GB
    nblk = e_pad // 128

    slot_start = np.zeros(GPC, dtype=np.int64)
    pos = 0
    for q in range(GPC):
        slot_start[q] = pos
        pos += int(S[q])

    # per-position segment index within its 128-block; per-block segment
    # program: list over blocks of (q, k, first_of_slot, last_of_slot)
    seg_of_pos = np.full(e_pad, -1, dtype=np.int64)
    blocks = [[] for _ in range(nblk)]
    for q in range(GPC):
        a = int(slot_start[q])
        b = a + int(S[q])
        if b == a:
            continue
        gb0, gb1 = a // 128, (b - 1) // 128
        for gb in range(gb0, gb1 + 1):
            lo = max(a, gb * 128)
            hi = min(b, (gb + 1) * 128)
            k = len(blocks[gb])
            seg_of_pos[lo:hi] = k
            blocks[gb].append(dict(
                q=q, k=k,
                first=(gb == gb0), last=(gb == gb1),
            ))
    max_k = max((len(b) for b in blocks), default=1)
    assert max_k * 128 <= 2040, f"segment offset overflow: {max_k}"

    slot_off = np.zeros(NCORES * GPC + 1, dtype=np.int64)
    np.cumsum(counts.reshape(-1), out=slot_off[1:])

    x_bf = np.zeros((NPAD, ATOM), dtype=BF16)
    x_bf[:N] = np.asarray(x, dtype=np.float32).astype(BF16)

    per_core = []
    for c in range(NCORES):
        xj = np.zeros((e_pad, ATOM), dtype=BF16)
        slb = np.full(e_pad, -1.0, dtype=np.float32)
        eat = np.zeros((e_pad, NBR), dtype=BF16)
        for q in range(GPC):
            kk = c * GPC + q
            a, b_ = slot_off[kk], slot_off[kk + 1]
            n = b_ - a
            if n == 0:
                continue
            o = int(slot_start[q])
            xj[o:o + n] = x_bf[dst_s[a:b_]]
            slb[o:o + n] = (src_s[a:b_] & 127).astype(np.float32) \
                + 128.0 * seg_of_pos[o:o + n]
            eat[o:o + n] = ea_s[a:b_]
        slb16 = slb.astype(FP16)
        # fused fp8 stream: A = [ea | xj] rows 0..191, packed for DoubleRow
        # as [96 partitions, 2 k-subtiles, e_pad]
        A = np.concatenate([eat.astype(np.float32),
                            xj.astype(np.float32)], axis=1).astype(FP8)
        axT = np.ascontiguousarray(
            A.T.reshape(2, 96, e_pad).transpose(1, 0, 2))
        per_core.append(
            dict(
                axT=axT,                                            # [96,2,e_pad]
                slbR=np.ascontiguousarray(slb16[None, :]),          # [1, e_pad]
                slbP=np.ascontiguousarray(
                    slb16.reshape(nblk, 128).T),                    # [128, nblk]
                xlocT=np.ascontiguousarray(
                    x_bf[c * NPC:(c + 1) * NPC].T),                 # [128, NPC]
            )
        )

    struct = dict(e_pad=e_pad, nblk=nblk, blocks=blocks, max_k=max_k)
    return per_core, struct


def _build(struct):
    """Build the 8-core SPMD bass program. Returns compiled Bacc."""
    e_pad = struct["e_pad"]
    nblk = struct["nblk"]
    blocks = struct["blocks"]
    max_k = max(2, struct["max_k"])

    nc = bacc.Bacc("TRN2", target_bir_lowering=False, debug=False,
                   num_devices=NCORES)
    f32, bf16, fp16 = _DT.float32, _DT.bfloat16, _DT.float16
    fp8 = _DT.float8e4

    def din(name, shape, dt):
        return nc.dram_tensor(name, shape, dt, kind="ExternalInput").ap()

    axT_d = din("axT", [96, 2, e_pad], fp8)
    slbR_d = din("slbR", [1, e_pad], fp16)
    slbP_d = din("slbP", [128, nblk], fp16)
    xlocT_d = din("xlocT", [128, NPC], bf16)
    wt_d = din("wt", [128, DOUT], bf16)          # W1 rows only
    wp_d = din("wp", [96, 2, DOUT], fp8)         # [W3; W2] DoubleRow-packed
    g1b1_d = din("g1b1", [128, 4], f32)
    g2b2_d = din("g2b2", [1, 2 * ATOM], f32)
    iotac_d = din("iotac", [128, max_k], f32)       # col n -> n + 128k
    iotar_d = din("iotar", [128, max_k * 128], fp16)  # row e -> n + 128k
    ones_c_d = din("ones_col", [128, 1], f32)
    ones_r_d = din("ones_row", [1, 128], f32)
    y_d = nc.dram_tensor("y", [NPC, ATOM], f32, kind="ExternalOutput").ap()

    AF = mybir.ActivationFunctionType
    ALU = mybir.AluOpType
    nbatch = e_pad // GB

    with tile.TileContext(nc) as tc:
        with (
            tc.tile_pool(name="const", bufs=1) as cp,
            tc.tile_pool(name="dram", bufs=1, space="DRAM") as dram,
        ):
            # persistent SBUF state
            w_sb = cp.tile([128, DOUT], bf16, tag="w")        # W1
            nc.scalar.dma_start(w_sb[:], wt_d[0:128, :])
            wp_sb = cp.tile([96, 2, DOUT], fp8, tag="wp")     # [W3; W2] packed
            nc.scalar.dma_start(wp_sb[:], wp_d[:])
            slbP_sb = cp.tile([128, nblk], fp16, tag="slbP")
            nc.scalar.dma_start(slbP_sb[:], slbP_d[:])
            iotac_sb = cp.tile([128, max_k], f32, tag="iotac")
            nc.scalar.dma_start(iotac_sb[:], iotac_d[:])
            iotar_sb = cp.tile([128, max_k * 128], fp16, tag="iotar")
            nc.scalar.dma_start(iotar_sb[:], iotar_d[:])
            g1b1_sb = cp.tile([128, 4], f32, tag="g1b1")
            nc.scalar.dma_start(g1b1_sb[:], g1b1_d[:])
            g2b2_sb = cp.tile([1, 2 * ATOM], f32, tag="g2b2")
            nc.scalar.dma_start(g2b2_sb[:], g2b2_d[:])
            ones_c = cp.tile([128, 1], f32, tag="onesc")
            nc.scalar.dma_start(ones_c[:], ones_c_d[:])
            ones_r = cp.tile([1, 128], f32, tag="onesr")
            nc.scalar.dma_start(ones_r[:], ones_r_d[:])

            summed = cp.tile([128, GPC * 128], f32, tag="summed")
            nc.vector.memset(summed[:], 0.0)

            mv_f = cp.tile([128, 2 * nbatch], f32, tag="mvf")
            mv_c = cp.tile([128, 2 * nbatch], f32, tag="mvc")
            s1 = cp.tile([128, 2], f32, tag="s1")
            t1 = cp.tile([128, 2], f32, tag="t1")

            spill_f = dram.tile([128, e_pad], bf16)
            spill_c = dram.tile([128, e_pad], bf16)

            # ---------------- PHASE 0: U = x_loc @ W1 ----------------
            # U_sb lives in a phase-0/1-scoped pool so its 24.5 KB/partition
            # is freed for phase 2.
            nchunk = GB // 512
            nbpb = GB // 128  # blocks per batch
            with (
                tc.tile_pool(name="u1", bufs=1) as u1p,
                tc.tile_pool(name="g1", bufs=3) as gp,
                tc.tile_pool(name="oh", bufs=3) as ohp,
                tc.tile_pool(name="st1", bufs=3) as sp,
                tc.tile_pool(name="ps1", bufs=2, space="PSUM") as pp,
            ):
                U_sb = u1p.tile([128, GPC * DOUT], bf16, tag="U")
                with tc.tile_pool(name="u0", bufs=2) as up:
                    xlocT_sb = up.tile([128, NPC], bf16, tag="xlT")
                    nc.scalar.dma_start(xlocT_sb[:], xlocT_d[:])
                    for q in range(GPC):
                        psU = pp.tile([128, GB], f32, name="ps0", tag="ps0")
                        nc.tensor.matmul(psU[:, 0:DOUT],
                                         xlocT_sb[:, q * 128:(q + 1) * 128],
                                         w_sb[:], start=True, stop=True)
                        nc.scalar.copy(U_sb[:, q * DOUT:(q + 1) * DOUT],
                                       psU[:, 0:DOUT])
                def load_batch(bi):
                    b0 = bi * GB
                    axb = gp.tile([96, 2, GB], fp8, name="axb", tag="axb")
                    nc.sync.dma_start(axb[:], axT_d[:, :, b0:b0 + GB])
                    slbb = gp.tile([128, GB], fp16, name="slbb", tag="slbb")
                    # partition-broadcast from the [1, e_pad] DRAM row
                    nc.sync.dma_start(
                        slbb[:],
                        slbR_d[:, b0:b0 + GB].to_broadcast([128, GB]))
                    return axb, slbb

                def build_ohs(bi, slbb):
                    segs_of = [blocks[bi * nbpb + c] or
                               [dict(q=0, k=0, first=False, last=False)]
                               for c in range(nbpb)]
                    kmax = max(len(sgl) for sgl in segs_of)
                    slb3 = slbb.rearrange("p (m l) -> p m l", l=128)
                    ohT = []
                    for k in range(kmax):
                        oh = ohp.tile([128, nbpb, 128], bf16,
                                      name=f"ohT{k}", tag=f"ohT{k}")
                        nc.vector.tensor_scalar(
                            oh[:], slb3, iotac_sb[:, k:k + 1], None,
                            ALU.is_equal)
                        ohT.append(oh)
                    return segs_of, ohT

                # software pipeline: one-hot builds run one batch ahead so
                # they execute before (not behind) the stats stall on DVE
                cur = load_batch(0)
                cur_ohs = build_ohs(0, cur[1])
                for bi in range(nbatch):
                    b0 = bi * GB
                    axb, slbb = cur
                    segs_of, ohT = cur_ohs
                    if bi + 1 < nbatch:
                        cur = load_batch(bi + 1)
                        cur_ohs = build_ohs(bi + 1, cur[1])

                    ps = [pp.tile([128, GB], f32, name=f"ps{h}", tag=f"ps{h}")
                          for h in range(2)]
                    for h in range(2):
                        # fused [ea|xj] @ [W3;W2]h — fp8 DoubleRow, one
                        # stationary for all chunks, 0.5 cyc/col
                        for p in range(nchunk):
                            nc.tensor.matmul(
                                ps[h][:, p * 512:(p + 1) * 512],
                                wp_sb[:, :, h * 128:(h + 1) * 128],
                                axb[:, :, p * 512:(p + 1) * 512],
                                start=True, stop=False,
                                perf_mode=mybir.MatmulPerfMode.DoubleRow)
                        # xi one-hot pieces: k>=1 singles, then k=0 merged
                        # runs (share one U stationary) carrying stop=True.
                        for p in range(nchunk):
                            for c in range(p * 4, p * 4 + 4):
                                for sg in segs_of[c][1:]:
                                    nc.tensor.matmul(
                                        ps[h][:, c * 128:(c + 1) * 128],
                                        U_sb[:, sg["q"] * DOUT + h * 128:
                                             sg["q"] * DOUT + (h + 1) * 128],
                                        ohT[sg["k"]][:, c, :],
                                        start=False, stop=False)
                            ca = p * 4
                            while ca < p * 4 + 4:
                                q0 = segs_of[ca][0]["q"]
                                cb = ca + 1
                                while cb < p * 4 + 4 and segs_of[cb][0]["q"] == q0:
                                    cb += 1
                                nc.tensor.matmul(
                                    ps[h][:, ca * 128:cb * 128],
                                    U_sb[:, q0 * DOUT + h * 128:
                                         q0 * DOUT + (h + 1) * 128],
                                    ohT[0][:, ca:cb, :],
                                    start=False, stop=True)
                                ca = cb

                    # psum -> bf16 stages on Act; stats via bn_stats on DVE
                    stage_f = sp.tile([128, GB], bf16, tag="stf")
                    stage_c = sp.tile([128, GB], bf16, tag="stc")
                    bst = sp.tile([128, 2, 12], f32, tag="bst")
                    for h, (stg, mv) in enumerate(
                            ((stage_f, mv_f), (stage_c, mv_c))):
                        nc.scalar.copy(stg[:], ps[h][:])
                        for p in range(GB // 512):
                            nc.vector.bn_stats(bst[:, h, p * 6:(p + 1) * 6],
                                               stg[:, p * 512:(p + 1) * 512])
                        nc.vector.bn_aggr(mv[:, 2 * bi:2 * bi + 2],
                                          bst[:, h, :])
                    nc.scalar.dma_start(spill_f[:, b0:b0 + GB], stage_f[:])
                    nc.scalar.dma_start(spill_c[:, b0:b0 + GB], stage_c[:])

            # ---------------- PHASE 2 ----------------
            with (
                tc.tile_pool(name="g2", bufs=3) as rp,
                tc.tile_pool(name="m2", bufs=2) as mp,
                tc.tile_pool(name="me", bufs=2) as ep,
                tc.tile_pool(name="oh2", bufs=2) as oh2,
                tc.tile_pool(name="psg", bufs=2, space="PSUM") as pg_pool,
                tc.tile_pool(name="sq2", bufs=1) as sq2p,
                tc.tile_pool(name="ps2", bufs=1, space="PSUM") as pq,
            ):
                mb_list = list(range(0, e_pad, MB))
                pairs = [mb_list[i:i + 2] for i in range(0, len(mb_list), 2)]

                def emit_load_ohs(m0):
                    msz = min(MB, e_pad - m0)
                    gf = rp.tile([128, MB], bf16, name="gf", tag="gf")
                    nc.sync.dma_start(gf[:, :msz], spill_f[:, m0:m0 + msz])
                    gc = rp.tile([128, MB], bf16, name="gc", tag="gc")
                    nc.sync.dma_start(gc[:, :msz], spill_c[:, m0:m0 + msz])
                    chunks = []
                    for c0 in range(0, msz // 128, 16):
                        gb0 = m0 // 128 + c0
                        nch = min(16, msz // 128 - c0)
                        kmax2 = max((len(blocks[gb0 + i]) for i in range(nch)),
                                    default=0)
                        ohs = []
                        for k in range(kmax2):
                            oneh = oh2.tile([128, 16, 128], bf16,
                                            name=f"oh{(c0 // 16) % 2}_{k}",
                                            tag=f"oh{(c0 // 16) % 2}_{k}")
                            nc.vector.tensor_tensor(
                                oneh[:, :nch, :],
                                iotar_sb[:, k * 128:(k + 1) * 128]
                                .rearrange("p (o l) -> p o l", o=1)
                                .to_broadcast([128, nch, 128]),
                                slbP_sb[:, gb0:gb0 + nch]
                                .to_broadcast([128, nch, 128]),
                                ALU.is_equal)
                            ohs.append(oneh)
                        chunks.append((c0, nch, ohs))
                    return gf, gc, chunks

                # prefetch pair 0 ahead of the stats barrier: its loads and
                # one-hot builds fill the collective round-trip gap
                prefetched = {m0: emit_load_ohs(m0) for m0 in pairs[0]}

                # ---------------- BN1 stats all-reduce ----------------
                # per-batch (mean, var) with equal counts (GB each, pads
                # zero): sum = GB*sum(means); sumsq = GB*sum(var + mean^2)
                st_loc = cp.tile([128, 4], f32, tag="stloc")
                ex2b = cp.tile([128, 2 * nbatch], f32, tag="ex2b")
                for h, mv in enumerate((mv_f, mv_c)):
                    means = mv.rearrange("p (n k) -> p k n", k=2)[:, 0, :]
                    varls = mv.rearrange("p (n k) -> p k n", k=2)[:, 1, :]
                    m2 = ex2b[:, h * nbatch:(h + 1) * nbatch]
                    nc.vector.tensor_tensor(m2, means, means, ALU.mult)
                    nc.vector.tensor_tensor(m2, m2, varls, ALU.add)
                    nc.vector.tensor_reduce(st_loc[:, h:h + 1], means,
                                            mybir.AxisListType.X, ALU.add)
                    nc.vector.tensor_reduce(st_loc[:, 2 + h:3 + h], m2,
                                            mybir.AxisListType.X, ALU.add)
                nc.vector.tensor_scalar_mul(st_loc[:], st_loc[:], float(GB))
                st_in = dram.tile([128, 4], f32)
                st_out = dram.tile([128, 4], f32)
                nc.gpsimd.dma_start(st_in[:], st_loc[:])
                nc.gpsimd.collective_compute(
                    "AllReduce", ALU.add, replica_groups=[list(range(NCORES))],
                    ins=[st_in.opt()], outs=[st_out.opt()],
                )
                st_g = cp.tile([128, 4], f32, tag="stg")
                nc.gpsimd.dma_start(st_g[:], st_out[:])
                # mean/var -> affine s1, t1  (b cancels in BN; never added)
                mv = cp.tile([128, 6], f32, tag="mv")
                nc.vector.tensor_scalar_mul(mv[:, 0:2], st_g[:, 0:2], 1.0 / E)
                nc.vector.tensor_scalar_mul(mv[:, 2:4], st_g[:, 2:4], 1.0 / E)
                nc.vector.tensor_tensor(mv[:, 4:6], mv[:, 0:2], mv[:, 0:2],
                                        ALU.mult)
                nc.vector.tensor_tensor(mv[:, 2:4], mv[:, 2:4], mv[:, 4:6],
                                        ALU.subtract)
                nc.vector.tensor_scalar_add(mv[:, 2:4], mv[:, 2:4],
                                            float(BN_EPS))
                std = cp.tile([128, 2], f32, tag="std")
                nc.scalar.activation(std[:], mv[:, 2:4], AF.Sqrt, bias=0.0)
                rstd = cp.tile([128, 2], f32, tag="rstd")
                nc.vector.reciprocal(rstd[:], std[:])
                nc.vector.tensor_tensor(s1[:], g1b1_sb[:, 0:2], rstd[:],
                                        ALU.mult)
                nc.vector.tensor_tensor(mv[:, 4:6], mv[:, 0:2], s1[:], ALU.mult)
                nc.vector.tensor_tensor(t1[:], g1b1_sb[:, 2:4], mv[:, 4:6],
                                        ALU.subtract)

                # BN2 running accumulators, filled as slots complete
                qred = sq2p.tile([128, 128], f32, name="qred", tag="qred")
                sqred = sq2p.tile([128, 128], f32, name="sqred", tag="sqred")
                sqtmp = sq2p.tile([128, 128], f32, name="sqtmp", tag="sqtmp")
                nc.vector.memset(qred[:], 0.0)
                nc.vector.memset(sqred[:], 0.0)

                ps_g = None
                for pair in pairs:
                    gfs, gcs, ohs_of, sigs, msgEs = {}, {}, {}, {}, {}
                    for m0 in pair:
                        gf, gc, chunks = prefetched.pop(m0, None) or \
                            emit_load_ohs(m0)
                        gfs[m0], gcs[m0], ohs_of[m0] = gf, gc, chunks
                    # all sigmoids (one act table), then all softplus
                    for m0 in pair:
                        msz = min(MB, e_pad - m0)
                        sig = mp.tile([128, MB], bf16, tag="sig")
                        nc.scalar.activation(sig[:, :msz], gfs[m0][:, :msz],
                                             AF.Sigmoid,
                                             bias=t1[:, 0:1], scale=s1[:, 0:1])
                        sigs[m0] = sig
                    # softplus = Ln(Exp(y) + 1); Exp and Ln share one act
                    # table set (natural_log_exp_and_others) so this whole
                    # group is a single table switch away from Sigmoid.
                    ecs = {}
                    for m0 in pair:
                        msz = min(MB, e_pad - m0)
                        ec = mp.tile([128, MB], bf16, tag="ec")
                        nc.scalar.activation(ec[:, :msz], gcs[m0][:, :msz],
                                             AF.Exp,
                                             bias=t1[:, 1:2], scale=s1[:, 1:2])
                        ecs[m0] = ec
                    for m0 in pair:
                        msz = min(MB, e_pad - m0)
                        nc.scalar.activation(gfs[m0][:, :msz], ecs[m0][:, :msz],
                                             AF.Ln, bias=1.0)
                    for m0 in pair:
                        msz = min(MB, e_pad - m0)
                        msgT = ecs[m0]  # reuse
                        nc.vector.tensor_tensor(msgT[:, :msz], sigs[m0][:, :msz],
                                                gfs[m0][:, :msz], ALU.mult)
                        msgE = ep.tile([128, MB // 128, 128], bf16, tag="msgE")
                        nc.sync.dma_start_transpose(msgE[:, :msz // 128, :],
                                                    msgT[:, :msz])
                        msgEs[m0] = msgE
                    for m0 in pair:
                        msz = min(MB, e_pad - m0)
                        for c0, nch, ohs in ohs_of[m0]:
                            for ci in range(nch):
                                gb = m0 // 128 + c0 + ci
                                for sg in blocks[gb]:
                                    if sg["first"]:
                                        ps_g = pg_pool.tile([128, 128], f32,
                                                            tag="psg")
                                    nc.tensor.matmul(
                                        ps_g[:], ohs[sg["k"]][:, ci, :],
                                        msgEs[m0][:, c0 + ci, :],
                                        start=sg["first"], stop=sg["last"])
                                    if sg["last"]:
                                        q = sg["q"]
                                        sm_q = summed[:, q * 128:(q + 1) * 128]
                                        nc.vector.tensor_tensor(
                                            sm_q, sm_q, ps_g[:], ALU.add)
                                        # BN2 running sums on the idle Pool
                                        # engine (SBUF-only, no broadcasts)
                                        nc.gpsimd.tensor_tensor(
                                            qred[:], qred[:], sm_q, ALU.add)
                                        nc.gpsimd.tensor_tensor(
                                            sqtmp[:], sm_q, sm_q, ALU.mult)
                                        nc.gpsimd.tensor_tensor(
                                            sqred[:], sqred[:], sqtmp[:],
                                            ALU.add)

                # ---------------- BN2 stats: fold partitions ----------------
                ps_st = pq.tile([1, 256], f32, tag="psst")
                nc.tensor.matmul(ps_st[:, 0:128], ones_c[:], qred[:],
                                 start=True, stop=True)
                nc.tensor.matmul(ps_st[:, 128:256], ones_c[:], sqred[:],
                                 start=True, stop=True)

                # ---------------- BN2 finalize ----------------
                st2 = cp.tile([1, 256], f32, tag="st2")
                nc.scalar.copy(st2[:], ps_st[:])
                st2_in = dram.tile([1, 256], f32)
                st2_out = dram.tile([1, 256], f32)
                nc.gpsimd.dma_start(st2_in[:], st2[:])
                nc.gpsimd.collective_compute(
                    "AllReduce", ALU.add, replica_groups=[list(range(NCORES))],
                    ins=[st2_in.opt()], outs=[st2_out.opt()],
                )
                st2g = cp.tile([1, 256], f32, tag="st2g")
                nc.gpsimd.dma_start(st2g[:], st2_out[:])
                mv2 = cp.tile([1, 384], f32, tag="mv2")
                nc.vector.tensor_scalar_mul(mv2[:, 0:256], st2g[:], 1.0 / N)
                nc.vector.tensor_tensor(mv2[:, 256:384], mv2[:, 0:128],
                                        mv2[:, 0:128], ALU.mult)
                nc.vector.tensor_tensor(mv2[:, 128:256], mv2[:, 128:256],
                                        mv2[:, 256:384], ALU.subtract)
                nc.vector.tensor_scalar_add(mv2[:, 128:256], mv2[:, 128:256],
                                            float(BN_EPS))
                std2 = cp.tile([1, 128], f32, tag="std2")
                nc.scalar.activation(std2[:], mv2[:, 128:256], AF.Sqrt, bias=0.0)
                rstd2 = cp.tile([1, 128], f32, tag="rstd2")
                nc.vector.reciprocal(rstd2[:], std2[:])
                strow = cp.tile([1, 256], f32, tag="strow")
                nc.vector.tensor_tensor(strow[:, 0:128], g2b2_sb[:, 0:128],
                                        rstd2[:], ALU.mult)
                nc.vector.tensor_tensor(mv2[:, 256:384], mv2[:, 0:128],
                                        strow[:, 0:128], ALU.mult)
                nc.vector.tensor_tensor(strow[:, 128:256], g2b2_sb[:, 128:256],
                                        mv2[:, 256:384], ALU.subtract)
                ps_bc = pq.tile([128, 256], f32, tag="psbc")
                nc.tensor.matmul(ps_bc[:], ones_r[:], strow[:], start=True, stop=True)
                s2t2 = cp.tile([128, 256], f32, tag="s2t2")
                nc.scalar.copy(s2t2[:], ps_bc[:])
                y3 = y_d.rearrange("(q p) f -> p q f", p=128)
                sm3 = summed.rearrange("p (q l) -> p q l", l=128)
                for q0 in range(0, GPC, 7):
                    og = sq2p.tile([128, 7, 128], f32, name="og", tag="og")
                    nc.vector.tensor_tensor(
                        og[:], sm3[:, q0:q0 + 7, :],
                        s2t2[:, 0:128].rearrange("p (o l) -> p o l", o=1)
                        .to_broadcast([128, 7, 128]), ALU.mult)
                    nc.vector.tensor_tensor(
                        og[:], og[:],
                        s2t2[:, 128:256].rearrange("p (o l) -> p o l", o=1)
                        .to_broadcast([128, 7, 128]), ALU.add)
                    nc.sync.dma_start(y3[:, q0:q0 + 7, :], og[:])
    nc.compile()
    return nc


def _make_in_maps(per_core, struct, inputs):
    max_k = max(2, struct["max_k"])
    g1 = np.asarray(inputs["gamma1"], np.float32).reshape(2, 128).T  # [128,2]
    b1 = np.asarray(inputs["beta1"], np.float32).reshape(2, 128).T
    g1b1 = np.ascontiguousarray(np.concatenate([g1, b1], axis=1))  # [128,4]
    g2b2 = np.concatenate([np.asarray(inputs["gamma2"], np.float32),
                           np.asarray(inputs["beta2"], np.float32)]).reshape(1, 256)
    iotac = (np.arange(128, dtype=np.float32)[:, None]
             + 128.0 * np.arange(max_k, dtype=np.float32)[None, :])
    iotar = np.tile(np.arange(max_k * 128, dtype=np.float32), (128, 1)).astype(FP16)
    W = np.asarray(inputs["W"], np.float32)
    Wstk = np.concatenate([W[256:320], W[128:256]], axis=0)   # [ea; xj] rows
    wp = np.ascontiguousarray(
        Wstk.reshape(2, 96, DOUT).transpose(1, 0, 2)).astype(FP8)
    shared = dict(
        wt=W[0:128].astype(BF16),
        wp=wp,
        g1b1=g1b1,
        g2b2=np.ascontiguousarray(g2b2),
        iotac=np.ascontiguousarray(iotac),
        iotar=np.ascontiguousarray(iotar),
        ones_col=np.ones((128, 1), np.float32),
        ones_row=np.ones((1, 128), np.float32),
    )
    return [{**pc, **shared} for pc in per_core]


def kernel(x, edge_index, edge_attr, W, b, gamma1, beta1, gamma2, beta2):
    per_core, struct = _prep(x, edge_index, edge_attr)
    in_maps = _make_in_maps(
        per_core, struct,
        dict(W=W, gamma1=gamma1, beta1=beta1, gamma2=gamma2, beta2=beta2),
    )
    nc = _build(struct)
    res = bass_utils.run_bass_kernel_spmd(nc, in_maps, core_ids=list(range(NCORES)))
    out = np.concatenate([res.results[c]["y"] for c in range(NCORES)], axis=0)
    return np.ascontiguousarray(out[:N])


if __name__ == "__main__":
    import reference

    inputs = {k: np.asarray(v) for k, v in reference.setup_inputs().items()}
    got = kernel(**inputs)
    exp = np.asarray(reference.reference(**inputs))
    err = np.abs(got - exp).max() / np.abs(exp).max()
    print("rel err:", err)


# revision 49
# speedup vs baseline: 1.0716x; 1.0716x over previous
"""CGConv (gnn_message_passing) Trainium2 kernel — 8-core SPMD, v3.

Strategy vs v2:
- x[dst] is gathered ON HOST into the sorted edge layout and streamed as a
  dense bf16 [128, e_pad] input (sequential DMA) — removes the SWDGE
  dma_gather whose GpSimd descriptor generation was 100%-busy for ~1ms.
- dst-half split dropped (only existed for int16 gather indices).
- slb row-replicate [128, e_pad] fp16 is built on host too: one-hot
  transposes come from a DVE is_equal against a streamed SBUF tile, killing
  all PE broadcast-transposes and the psum ping-pong they caused.
- Phase-1 batches of 1024 edges with double-buffered PSUM (2 halves x 2
  bufs x 2 banks = 8 banks) so psum drains overlap the next batch matmuls.
- BN1 sums come free: Act Copy accum_out (h0) + Pool scalar_tensor_tensor
  copy (h1); sum-of-squares via DVE scalar_tensor_tensor on bf16 stages.
- Phase 2 uses the native Softplus activation (no Exp+Ln) and groups
  activations per table set to minimize ACT_TABLE_LOAD switches.
- BN2 stats moved out of the scatter into a tiny end pass over `summed`.
"""

import sys

sys.path.insert(0, "/opt/trn_rl_repo")

import numpy as np
import ml_dtypes

from concourse import bass, bacc, tile, mybir
from concourse import bass_utils

BF16 = ml_dtypes.bfloat16
FP16 = np.float16
FP8 = ml_dtypes.float8_e4m3

# Problem constants (hardcoded per harness contract)
N, E, ATOM, NBR = 50000, 800000, 128, 64
DOUT = 2 * ATOM  # 256
BN_EPS = 1e-5

NCORES = 8
GPC = 49                      # groups of 128 nodes per core
NPC = GPC * 128               # 6272 nodes per core
NPAD = NCORES * NPC           # 50176
GB = 1024                     # phase-1 batch (edges)
MB = 4096                     # phase-2 mega-batch (edges)

_DT = mybir.dt


def _ceil(a, b):
    return -(-a // b)


def _prep(x, edge_index, edge_attr):
    """Host-side sharding: sort edges by (src core, src group), gather x[dst]
    into that layout, pad slots to uniform-across-cores sizes."""
    src = np.asarray(edge_index[0], dtype=np.int64)
    dst = np.asarray(edge_index[1], dtype=np.int64)
    ea = np.asarray(edge_attr, dtype=np.float32)

    g = src >> 7                      # node group 0..391
    core = g // GPC
    gi = g % GPC
    key = core * GPC + gi
    order = np.argsort(key, kind="stable")
    src_s, dst_s = src[order], dst[order]
    ea_s = ea[order].astype(BF16)

    counts = np.bincount(key, minlength=NCORES * GPC).reshape(NCORES, GPC)
    S = counts.max(axis=0)                      # [GPC] uniform slot sizes
    e_pad = _ceil(max(int(S.sum()), 1), GB) * GB
    nblk = e_pad // 128

    slot_start = np.zeros(GPC, dtype=np.int64)
    pos = 0
    for q in range(GPC):
        slot_start[q] = pos
        pos += int(S[q])

    # per-position segment index within its 128-block; per-block segment
    # program: list over blocks of (q, k, first_of_slot, last_of_slot)
    seg_of_pos = np.full(e_pad, -1, dtype=np.int64)
    blocks = [[] for _ in range(nblk)]
    for q in range(GPC):
        a = int(slot_start[q])
        b = a + int(S[q])
        if b == a:
            continue
        gb0, gb1 = a // 128, (b - 1) // 128
        for gb in range(gb0, gb1 + 1):
            lo = max(a, gb * 128)
            hi = min(b, (gb + 1) * 128)
            k = len(blocks[gb])
            seg_of_pos[lo:hi] = k
            blocks[gb].append(dict(
                q=q, k=k,
                first=(gb == gb0), last=(gb == gb1),
            ))
    max_k = max((len(b) for b in blocks), default=1)
    assert max_k * 128 <= 2040, f"segment offset overflow: {max_k}"

    slot_off = np.zeros(NCORES * GPC + 1, dtype=np.int64)
    np.cumsum(counts.reshape(-1), out=slot_off[1:])

    x_bf = np.zeros((NPAD, ATOM), dtype=BF16)
    x_bf[:N] = np.asarray(x, dtype=np.float32).astype(BF16)

    per_core = []
    for c in range(NCORES):
        xj = np.zeros((e_pad, ATOM), dtype=BF16)
        slb = np.full(e_pad, -1.0, dtype=np.float32)
        eat = np.zeros((e_pad, NBR), dtype=BF16)
        for q in range(GPC):
            kk = c * GPC + q
            a, b_ = slot_off[kk], slot_off[kk + 1]
            n = b_ - a
            if n == 0:
                continue
            o = int(slot_start[q])
            xj[o:o + n] = x_bf[dst_s[a:b_]]
            slb[o:o + n] = (src_s[a:b_] & 127).astype(np.float32) \
                + 128.0 * seg_of_pos[o:o + n]
            eat[o:o + n] = ea_s[a:b_]
        slb16 = slb.astype(FP16)
        # fused fp8 stream: A = [ea | xj] rows 0..191, packed for DoubleRow
        # as [96 partitions, 2 k-subtiles, e_pad]
        A = np.concatenate([eat.astype(np.float32),
                            xj.astype(np.float32)], axis=1).astype(FP8)
        axT = np.ascontiguousarray(
            A.T.reshape(2, 96, e_pad).transpose(1, 0, 2))
        per_core.append(
            dict(
                axT=axT,                                            # [96,2,e_pad]
                slbR=np.ascontiguousarray(
                    np.broadcast_to(slb16[None, :], (128, e_pad))),  # [128,e_pad]
                slbP=np.ascontiguousarray(
                    slb16.reshape(nblk, 128).T),                    # [128, nblk]
                xlocT=np.ascontiguousarray(
                    x_bf[c * NPC:(c + 1) * NPC].T),                 # [128, NPC]
            )
        )

    struct = dict(e_pad=e_pad, nblk=nblk, blocks=blocks, max_k=max_k)
    return per_core, struct


def _build(struct):
    """Build the 8-core SPMD bass program. Returns compiled Bacc."""
    e_pad = struct["e_pad"]
    nblk = struct["nblk"]
    blocks = struct["blocks"]
    max_k = max(2, struct["max_k"])

    nc = bacc.Bacc("TRN2", target_bir_lowering=False, debug=False,
                   num_devices=NCORES)
    f32, bf16, fp16 = _DT.float32, _DT.bfloat16, _DT.float16
    fp8 = _DT.float8e4

    def din(name, shape, dt):
        return nc.dram_tensor(name, shape, dt, kind="ExternalInput").ap()

    axT_d = din("axT", [96, 2, e_pad], fp8)
    slbR_d = din("slbR", [128, e_pad], fp16)
    slbP_d = din("slbP", [128, nblk], fp16)
    xlocT_d = din("xlocT", [128, NPC], bf16)
    wt_d = din("wt", [128, DOUT], bf16)          # W1 rows only
    wp_d = din("wp", [96, 2, DOUT], fp8)         # [W3; W2] DoubleRow-packed
    g1b1_d = din("g1b1", [128, 4], f32)
    g2b2_d = din("g2b2", [1, 2 * ATOM], f32)
    iotac_d = din("iotac", [128, max_k], f32)       # col n -> n + 128k
    iotar_d = din("iotar", [128, max_k * 128], fp16)  # row e -> n + 128k
    ones_c_d = din("ones_col", [128, 1], f32)
    ones_r_d = din("ones_row", [1, 128], f32)
    y_d = nc.dram_tensor("y", [NPC, ATOM], f32, kind="ExternalOutput").ap()

    AF = mybir.ActivationFunctionType
    ALU = mybir.AluOpType
    nbatch = e_pad // GB

    with tile.TileContext(nc) as tc:
        with (
            tc.tile_pool(name="const", bufs=1) as cp,
            tc.tile_pool(name="dram", bufs=1, space="DRAM") as dram,
        ):
            # persistent SBUF state
            w_sb = cp.tile([128, DOUT], bf16, tag="w")        # W1
            nc.scalar.dma_start(w_sb[:], wt_d[0:128, :])
            wp_sb = cp.tile([96, 2, DOUT], fp8, tag="wp")     # [W3; W2] packed
            nc.scalar.dma_start(wp_sb[:], wp_d[:])
            slbP_sb = cp.tile([128, nblk], fp16, tag="slbP")
            nc.scalar.dma_start(slbP_sb[:], slbP_d[:])
            iotac_sb = cp.tile([128, max_k], f32, tag="iotac")
            nc.scalar.dma_start(iotac_sb[:], iotac_d[:])
            iotar_sb = cp.tile([128, max_k * 128], fp16, tag="iotar")
            nc.scalar.dma_start(iotar_sb[:], iotar_d[:])
            g1b1_sb = cp.tile([128, 4], f32, tag="g1b1")
            nc.scalar.dma_start(g1b1_sb[:], g1b1_d[:])
            g2b2_sb = cp.tile([1, 2 * ATOM], f32, tag="g2b2")
            nc.scalar.dma_start(g2b2_sb[:], g2b2_d[:])
            ones_c = cp.tile([128, 1], f32, tag="onesc")
            nc.scalar.dma_start(ones_c[:], ones_c_d[:])
            ones_r = cp.tile([1, 128], f32, tag="onesr")
            nc.scalar.dma_start(ones_r[:], ones_r_d[:])

            summed = cp.tile([128, GPC * 128], f32, tag="summed")
            nc.vector.memset(summed[:], 0.0)

            mv_f = cp.tile([128, 2 * nbatch], f32, tag="mvf")
            mv_c = cp.tile([128, 2 * nbatch], f32, tag="mvc")
            s1 = cp.tile([128, 2], f32, tag="s1")
            t1 = cp.tile([128, 2], f32, tag="t1")

            spill_f = dram.tile([128, e_pad], bf16)
            spill_c = dram.tile([128, e_pad], bf16)

            # ---------------- PHASE 0: U = x_loc @ W1 ----------------
            # U_sb lives in a phase-0/1-scoped pool so its 24.5 KB/partition
            # is freed for phase 2.
            nchunk = GB // 512
            nbpb = GB // 128  # blocks per batch
            with (
                tc.tile_pool(name="u1", bufs=1) as u1p,
                tc.tile_pool(name="g1", bufs=3) as gp,
                tc.tile_pool(name="oh", bufs=3) as ohp,
                tc.tile_pool(name="st1", bufs=3) as sp,
                tc.tile_pool(name="ps1", bufs=2, space="PSUM") as pp,
            ):
                U_sb = u1p.tile([128, GPC * DOUT], bf16, tag="U")
                with tc.tile_pool(name="u0", bufs=2) as up:
                    xlocT_sb = up.tile([128, NPC], bf16, tag="xlT")
                    nc.scalar.dma_start(xlocT_sb[:], xlocT_d[:])
                    for q in range(GPC):
                        psU = pp.tile([128, GB], f32, name="ps0", tag="ps0")
                        nc.tensor.matmul(psU[:, 0:DOUT],
                                         xlocT_sb[:, q * 128:(q + 1) * 128],
                                         w_sb[:], start=True, stop=True)
                        nc.scalar.copy(U_sb[:, q * DOUT:(q + 1) * DOUT],
                                       psU[:, 0:DOUT])
                def load_batch(bi):
                    b0 = bi * GB
                    axb = gp.tile([96, 2, GB], fp8, name="axb", tag="axb")
                    nc.sync.dma_start(axb[:], axT_d[:, :, b0:b0 + GB])
                    slbb = gp.tile([128, GB], fp16, name="slbb", tag="slbb")
                    nc.sync.dma_start(slbb[:], slbR_d[:, b0:b0 + GB])
                    return axb, slbb

                def build_ohs(bi, slbb):
                    segs_of = [blocks[bi * nbpb + c] or
                               [dict(q=0, k=0, first=False, last=False)]
                               for c in range(nbpb)]
                    kmax = max(len(sgl) for sgl in segs_of)
                    slb3 = slbb.rearrange("p (m l) -> p m l", l=128)
                    ohT = []
                    for k in range(kmax):
                        oh = ohp.tile([128, nbpb, 128], bf16,
                                      name=f"ohT{k}", tag=f"ohT{k}")
                        nc.vector.tensor_scalar(
                            oh[:], slb3, iotac_sb[:, k:k + 1], None,
                            ALU.is_equal)
                        ohT.append(oh)
                    return segs_of, ohT

                # software pipeline: one-hot builds run one batch ahead so
                # they execute before (not behind) the stats stall on DVE
                cur = load_batch(0)
                cur_ohs = build_ohs(0, cur[1])
                for bi in range(nbatch):
                    b0 = bi * GB
                    axb, slbb = cur
                    segs_of, ohT = cur_ohs
                    if bi + 1 < nbatch:
                        cur = load_batch(bi + 1)
                        cur_ohs = build_ohs(bi + 1, cur[1])

                    ps = [pp.tile([128, GB], f32, name=f"ps{h}", tag=f"ps{h}")
                          for h in range(2)]
                    for h in range(2):
                        # fused [ea|xj] @ [W3;W2]h — fp8 DoubleRow, one
                        # stationary for all chunks, 0.5 cyc/col
                        for p in range(nchunk):
                            nc.tensor.matmul(
                                ps[h][:, p * 512:(p + 1) * 512],
                                wp_sb[:, :, h * 128:(h + 1) * 128],
                                axb[:, :, p * 512:(p + 1) * 512],
                                start=True, stop=False,
                                perf_mode=mybir.MatmulPerfMode.DoubleRow)
                        # xi one-hot pieces: k>=1 singles, then k=0 merged
                        # runs (share one U stationary) carrying stop=True.
                        for p in range(nchunk):
                            for c in range(p * 4, p * 4 + 4):
                                for sg in segs_of[c][1:]:
                                    nc.tensor.matmul(
                                        ps[h][:, c * 128:(c + 1) * 128],
                                        U_sb[:, sg["q"] * DOUT + h * 128:
                                             sg["q"] * DOUT + (h + 1) * 128],
                                        ohT[sg["k"]][:, c, :],
                                        start=False, stop=False)
                            ca = p * 4
                            while ca < p * 4 + 4:
                                q0 = segs_of[ca][0]["q"]
                                cb = ca + 1
                                while cb < p * 4 + 4 and segs_of[cb][0]["q"] == q0:
                                    cb += 1
                                nc.tensor.matmul(
                                    ps[h][:, ca * 128:cb * 128],
                                    U_sb[:, q0 * DOUT + h * 128:
                                         q0 * DOUT + (h + 1) * 128],
                                    ohT[0][:, ca:cb, :],
                                    start=False, stop=True)
                                ca = cb

                    # psum -> bf16 stages on Act; stats via bn_stats on DVE
                    stage_f = sp.tile([128, GB], bf16, tag="stf")
                    stage_c = sp.tile([128, GB], bf16, tag="stc")
                    bst = sp.tile([128, 2, 12], f32, tag="bst")
                    for h, (stg, mv) in enumerate(
                            ((stage_f, mv_f), (stage_c, mv_c))):
                        nc.scalar.copy(stg[:], ps[h][:])
                        for p in range(GB // 512):
                            nc.vector.bn_stats(bst[:, h, p * 6:(p + 1) * 6],
                                               stg[:, p * 512:(p + 1) * 512])
                        nc.vector.bn_aggr(mv[:, 2 * bi:2 * bi + 2],
                                          bst[:, h, :])
                    nc.scalar.dma_start(spill_f[:, b0:b0 + GB], stage_f[:])
                    nc.scalar.dma_start(spill_c[:, b0:b0 + GB], stage_c[:])

            # ---------------- PHASE 2 ----------------
            with (
                tc.tile_pool(name="g2", bufs=3) as rp,
                tc.tile_pool(name="m2", bufs=2) as mp,
                tc.tile_pool(name="me", bufs=2) as ep,
                tc.tile_pool(name="oh2", bufs=2) as oh2,
                tc.tile_pool(name="psg", bufs=2, space="PSUM") as pg_pool,
                tc.tile_pool(name="sq2", bufs=1) as sq2p,
                tc.tile_pool(name="ps2", bufs=1, space="PSUM") as pq,
            ):
                mb_list = list(range(0, e_pad, MB))
                pairs = [mb_list[i:i + 2] for i in range(0, len(mb_list), 2)]

                def emit_load_ohs(m0):
                    msz = min(MB, e_pad - m0)
                    gf = rp.tile([128, MB], bf16, name="gf", tag="gf")
                    nc.sync.dma_start(gf[:, :msz], spill_f[:, m0:m0 + msz])
                    gc = rp.tile([128, MB], bf16, name="gc", tag="gc")
                    nc.sync.dma_start(gc[:, :msz], spill_c[:, m0:m0 + msz])
                    chunks = []
                    for c0 in range(0, msz // 128, 16):
                        gb0 = m0 // 128 + c0
                        nch = min(16, msz // 128 - c0)
                        kmax2 = max((len(blocks[gb0 + i]) for i in range(nch)),
                                    default=0)
                        ohs = []
                        for k in range(kmax2):
                            oneh = oh2.tile([128, 16, 128], bf16,
                                            name=f"oh{(c0 // 16) % 2}_{k}",
                                            tag=f"oh{(c0 // 16) % 2}_{k}")
                            nc.vector.tensor_tensor(
                                oneh[:, :nch, :],
                                iotar_sb[:, k * 128:(k + 1) * 128]
                                .rearrange("p (o l) -> p o l", o=1)
                                .to_broadcast([128, nch, 128]),
                                slbP_sb[:, gb0:gb0 + nch]
                                .to_broadcast([128, nch, 128]),
                                ALU.is_equal)
                            ohs.append(oneh)
                        chunks.append((c0, nch, ohs))
                    return gf, gc, chunks

                # prefetch pair 0 ahead of the stats barrier: its loads and
                # one-hot builds fill the collective round-trip gap
                prefetched = {m0: emit_load_ohs(m0) for m0 in pairs[0]}

                # ---------------- BN1 stats all-reduce ----------------
                # per-batch (mean, var) with equal counts (GB each, pads
                # zero): sum = GB*sum(means); sumsq = GB*sum(var + mean^2)
                st_loc = cp.tile([128, 4], f32, tag="stloc")
                ex2b = cp.tile([128, 2 * nbatch], f32, tag="ex2b")
                for h, mv in enumerate((mv_f, mv_c)):
                    means = mv.rearrange("p (n k) -> p k n", k=2)[:, 0, :]
                    varls = mv.rearrange("p (n k) -> p k n", k=2)[:, 1, :]
                    m2 = ex2b[:, h * nbatch:(h + 1) * nbatch]
                    nc.vector.tensor_tensor(m2, means, means, ALU.mult)
                    nc.vector.tensor_tensor(m2, m2, varls, ALU.add)
                    nc.vector.tensor_reduce(st_loc[:, h:h + 1], means,
                                            mybir.AxisListType.X, ALU.add)
                    nc.vector.tensor_reduce(st_loc[:, 2 + h:3 + h], m2,
                                            mybir.AxisListType.X, ALU.add)
                nc.vector.tensor_scalar_mul(st_loc[:], st_loc[:], float(GB))
                st_in = dram.tile([128, 4], f32)
                st_out = dram.tile([128, 4], f32)
                nc.gpsimd.dma_start(st_in[:], st_loc[:])
                nc.gpsimd.collective_compute(
                    "AllReduce", ALU.add, replica_groups=[list(range(NCORES))],
                    ins=[st_in.opt()], outs=[st_out.opt()],
                )
                st_g = cp.tile([128, 4], f32, tag="stg")
                nc.gpsimd.dma_start(st_g[:], st_out[:])
                # mean/var -> affine s1, t1  (b cancels in BN; never added)
                mv = cp.tile([128, 6], f32, tag="mv")
                nc.vector.tensor_scalar_mul(mv[:, 0:2], st_g[:, 0:2], 1.0 / E)
                nc.vector.tensor_scalar_mul(mv[:, 2:4], st_g[:, 2:4], 1.0 / E)
                nc.vector.tensor_tensor(mv[:, 4:6], mv[:, 0:2], mv[:, 0:2],
                                        ALU.mult)
                nc.vector.tensor_tensor(mv[:, 2:4], mv[:, 2:4], mv[:, 4:6],
                                        ALU.subtract)
                nc.vector.tensor_scalar_add(mv[:, 2:4], mv[:, 2:4],
                                            float(BN_EPS))
                std = cp.tile([128, 2], f32, tag="std")
                nc.scalar.activation(std[:], mv[:, 2:4], AF.Sqrt, bias=0.0)
                rstd = cp.tile([128, 2], f32, tag="rstd")
                nc.vector.reciprocal(rstd[:], std[:])
                nc.vector.tensor_tensor(s1[:], g1b1_sb[:, 0:2], rstd[:],
                                        ALU.mult)
                nc.vector.tensor_tensor(mv[:, 4:6], mv[:, 0:2], s1[:], ALU.mult)
                nc.vector.tensor_tensor(t1[:], g1b1_sb[:, 2:4], mv[:, 4:6],
                                        ALU.subtract)

                # BN2 running accumulators, filled as slots complete
                qred = sq2p.tile([128, 128], f32, name="qred", tag="qred")
                sqred = sq2p.tile([128, 128], f32, name="sqred", tag="sqred")
                sqtmp = sq2p.tile([128, 128], f32, name="sqtmp", tag="sqtmp")
                nc.vector.memset(qred[:], 0.0)
                nc.vector.memset(sqred[:], 0.0)

                ps_g = None
                for pair in pairs:
                    gfs, gcs, ohs_of, sigs, msgEs = {}, {}, {}, {}, {}
                    for m0 in pair:
                        gf, gc, chunks = prefetched.pop(m0, None) or \
                            emit_load_ohs(m0)
                        gfs[m0], gcs[m0], ohs_of[m0] = gf, gc, chunks
                    # all sigmoids (one act table), then all softplus
                    for m0 in pair:
                        msz = min(MB, e_pad - m0)
                        sig = mp.tile([128, MB], bf16, tag="sig")
                        nc.scalar.activation(sig[:, :msz], gfs[m0][:, :msz],
                                             AF.Sigmoid,
                                             bias=t1[:, 0:1], scale=s1[:, 0:1])
                        sigs[m0] = sig
                    # softplus = Ln(Exp(y) + 1); Exp and Ln share one act
                    # table set (natural_log_exp_and_others) so this whole
                    # group is a single table switch away from Sigmoid.
                    ecs = {}
                    for m0 in pair:
                        msz = min(MB, e_pad - m0)
                        ec = mp.tile([128, MB], bf16, tag="ec")
                        nc.scalar.activation(ec[:, :msz], gcs[m0][:, :msz],
                                             AF.Exp,
                                             bias=t1[:, 1:2], scale=s1[:, 1:2])
                        ecs[m0] = ec
                    for m0 in pair:
                        msz = min(MB, e_pad - m0)
                        nc.scalar.activation(gfs[m0][:, :msz], ecs[m0][:, :msz],
                                             AF.Ln, bias=1.0)
                    for m0 in pair:
                        msz = min(MB, e_pad - m0)
                        msgT = ecs[m0]  # reuse
                        nc.vector.tensor_tensor(msgT[:, :msz], sigs[m0][:, :msz],
                                                gfs[m0][:, :msz], ALU.mult)
                        msgE = ep.tile([128, MB // 128, 128], bf16, tag="msgE")
                        nc.sync.dma_start_transpose(msgE[:, :msz // 128, :],
                                                    msgT[:, :msz])
                        msgEs[m0] = msgE
                    for m0 in pair:
                        msz = min(MB, e_pad - m0)
                        for c0, nch, ohs in ohs_of[m0]:
                            for ci in range(nch):
                                gb = m0 // 128 + c0 + ci
                                for sg in blocks[gb]:
                                    if sg["first"]:
                                        ps_g = pg_pool.tile([128, 128], f32,
                                                            tag="psg")
                                    nc.tensor.matmul(
                                        ps_g[:], ohs[sg["k"]][:, ci, :],
                                        msgEs[m0][:, c0 + ci, :],
                                        start=sg["first"], stop=sg["last"])
                                    if sg["last"]:
                                        q = sg["q"]
                                        sm_q = summed[:, q * 128:(q + 1) * 128]
                                        nc.vector.tensor_tensor(
                                            sm_q, sm_q, ps_g[:], ALU.add)
                                        # BN2 running sums on the idle Pool
                                        # engine (SBUF-only, no broadcasts)
                                        nc.gpsimd.tensor_tensor(
                                            qred[:], qred[:], sm_q, ALU.add)
                                        nc.gpsimd.tensor_tensor(
                                            sqtmp[:], sm_q, sm_q, ALU.mult)
                                        nc.gpsimd.tensor_tensor(
                                            sqred[:], sqred[:], sqtmp[:],
                                            ALU.add)

                # ---------------- BN2 stats: fold partitions ----------------
                ps_st = pq.tile([1, 256], f32, tag="psst")
                nc.tensor.matmul(ps_st[:, 0:128], ones_c[:], qred[:],
                                 start=True, stop=True)
                nc.tensor.matmul(ps_st[:, 128:256], ones_c[:], sqred[:],
                                 start=True, stop=True)

                # ---------------- BN2 finalize ----------------
                st2 = cp.tile([1, 256], f32, tag="st2")
                nc.scalar.copy(st2[:], ps_st[:])
                st2_in = dram.tile([1, 256], f32)
                st2_out = dram.tile([1, 256], f32)
                nc.gpsimd.dma_start(st2_in[:], st2[:])
                nc.gpsimd.collective_compute(
                    "AllReduce", ALU.add, replica_groups=[list(range(NCORES))],
                    ins=[st2_in.opt()], outs=[st2_out.opt()],
                )
                st2g = cp.tile([1, 256], f32, tag="st2g")
                nc.gpsimd.dma_start(st2g[:], st2_out[:])
                mv2 = cp.tile([1, 384], f32, tag="mv2")
                nc.vector.tensor_scalar_mul(mv2[:, 0:256], st2g[:], 1.0 / N)
                nc.vector.tensor_tensor(mv2[:, 256:384], mv2[:, 0:128],
                                        mv2[:, 0:128], ALU.mult)
                nc.vector.tensor_tensor(mv2[:, 128:256], mv2[:, 128:256],
                                        mv2[:, 256:384], ALU.subtract)
                nc.vector.tensor_scalar_add(mv2[:, 128:256], mv2[:, 128:256],
                                            float(BN_EPS))
                std2 = cp.tile([1, 128], f32, tag="std2")
                nc.scalar.activation(std2[:], mv2[:, 128:256], AF.Sqrt, bias=0.0)
                rstd2 = cp.tile([1, 128], f32, tag="rstd2")
                nc.vector.reciprocal(rstd2[:], std2[:])
                strow = cp.tile([1, 256], f32, tag="strow")
                nc.vector.tensor_tensor(strow[:, 0:128], g2b2_sb[:, 0:128],
                                        rstd2[:], ALU.mult)
                nc.vector.tensor_tensor(mv2[:, 256:384], mv2[:, 0:128],
                                        strow[:, 0:128], ALU.mult)
                nc.vector.tensor_tensor(strow[:, 128:256], g2b2_sb[:, 128:256],
                                        mv2[:, 256:384], ALU.subtract)
                ps_bc = pq.tile([128, 256], f32, tag="psbc")
                nc.tensor.matmul(ps_bc[:], ones_r[:], strow[:], start=True, stop=True)
                s2t2 = cp.tile([128, 256], f32, tag="s2t2")
                nc.scalar.copy(s2t2[:], ps_bc[:])
                y3 = y_d.rearrange("(q p) f -> p q f", p=128)
                sm3 = summed.rearrange("p (q l) -> p q l", l=128)
                for q0 in range(0, GPC, 7):
                    og = sq2p.tile([128, 7, 128], f32, name="og", tag="og")
                    nc.vector.tensor_tensor(
                        og[:], sm3[:, q0:q0 + 7, :],
                        s2t2[:, 0:128].rearrange("p (o l) -> p o l", o=1)
                        .to_broadcast([128, 7, 128]), ALU.mult)
                    nc.vector.tensor_tensor(
                        og[:], og[:],
                        s2t2[:, 128:256].rearrange("p (o l) -> p o l", o=1)
                        .to_broadcast([128, 7, 128]), ALU.add)
                    nc.sync.dma_start(y3[:, q0:q0 + 7, :], og[:])
    nc.compile()
    return nc


def _make_in_maps(per_core, struct, inputs):
    max_k = max(2, struct["max_k"])
    g1 = np.asarray(inputs["gamma1"], np.float32).reshape(2, 128).T  # [128,2]
    b1 = np.asarray(inputs["beta1"], np.float32).reshape(2, 128).T
    g1b1 = np.ascontiguousarray(np.concatenate([g1, b1], axis=1))  # [128,4]
    g2b2 = np.concatenate([np.asarray(inputs["gamma2"], np.float32),
                           np.asarray(inputs["beta2"], np.float32)]).reshape(1, 256)
    iotac = (np.arange(128, dtype=np.float32)[:, None]
             + 128.0 * np.arange(max_k, dtype=np.float32)[None, :])
    iotar = np.tile(np.arange(max_k * 128, dtype=np.float32), (128, 1)).astype(FP16)
    W = np.asarray(inputs["W"], np.float32)
    Wstk = np.concatenate([W[256:320], W[128:256]], axis=0)   # [ea; xj] rows
    wp = np.ascontiguousarray(
        Wstk.reshape(2, 96, DOUT).transpose(1, 0, 2)).astype(FP8)
    shared = dict(
        wt=W[0:128].astype(BF16),
        wp=wp,
        g1b1=g1b1,
        g2b2=np.ascontiguousarray(g2b2),
        iotac=np.ascontiguousarray(iotac),
        iotar=np.ascontiguousarray(iotar),
        ones_col=np.ones((128, 1), np.float32),
        ones_row=np.ones((1, 128), np.float32),
    )
    return [{**pc, **shared} for pc in per_core]


def kernel(x, edge_index, edge_attr, W, b, gamma1, beta1, gamma2, beta2):
    per_core, struct = _prep(x, edge_index, edge_attr)
    in_maps = _make_in_maps(
        per_core, struct,
        dict(W=W, gamma1=gamma1, beta1=beta1, gamma2=gamma2, beta2=beta2),
    )
    nc = _build(struct)
    res = bass_utils.run_bass_kernel_spmd(nc, in_maps, core_ids=list(range(NCORES)))
    out = np.concatenate([res.results[c]["y"] for c in range(NCORES)], axis=0)
    return np.ascontiguousarray(out[:N])


if __name__ == "__main__":
    import reference

    inputs = {k: np.asarray(v) for k, v in reference.setup_inputs().items()}
    got = kernel(**inputs)
    exp = np.asarray(reference.reference(**inputs))
    err = np.abs(got - exp).max() / np.abs(exp).max()
    print("rel err:", err)
